# revision 71
# baseline (speedup 1.0000x reference)
"""Trainium2 Bass kernel for the 8-head causal "transposed-softmax" attention.

Math (per head n, batch b), with x: [S, E], Wq/Wk/Wvin: [E, D], Wvout: [D, E]:
    Q = x @ Wq ; K = x @ Wk ; V = x @ Wvin                     # [S, D]
    P[r, c] = softmax_c( mask_{c<=r}( K[r] . Q[c] ) )          # [S, S]
    out    += (P @ V) @ Wvout                                  # summed over heads

Sharding: 8 cores = 4 batches x 2 head-groups (4 heads each); identical SPMD
program per core; the two head-group partials per batch are summed on host.

Design notes (cost-model-driven; TimelineSim charges ops by free-size only):
  - scores are built TRANSPOSED (S_T[c, r]) so exp(S_T) is directly the
    stationary operand of the context matmul.
  - per-row softmax shift rides the scores matmul as a 65th contraction row
    (Q' row 64 = 1, K' row 64 = -(sampled_rowmax + MARGIN - 127*ln2)).
    The +127*ln2 pre-bias makes the DVE fast-exp a single tensor_scalar:
        E = bitcast_bf16(int16(round(max(s', 0) * 128/ln2)))
    which equals exp(s - shift) exactly in bf16 grid (Schraudolph), with
    underflow clamped to +0. ACT spans compute exp(s' - 127*ln2) via the
    constant bias operand. exp work is split ACT/DVE to balance engines.
  - row-max prepass samples the first 64 causal columns (exact masked diag
    for tile 0); validated worst miss ~96 < 129 (int16 overflow bound).
  - Q/K for the two heads of a pair are projected in one matmul; the halves
    are re-based to per-head [65, S] tiles via SBUF->SBUF DMA (partition
    shifts are free on DMA queues, expensive on engines).
  - context accumulates per r-tile in 7/7/2-tile PSUM chunks (65 cols incl.
    a ones-denominator column); normalization is one reciprocal + one
    broadcast tensor_tensor per chunk.
  - per-pair [128,128] transposes move normalized context into ctxT2 so the
    output projection contracts a full 128-deep head pair per matmul.
  - PE work is emitted interleaved (scores spans x ctx/prepass/transpose
    fillers) to keep the PE continuously busy: the cost model rewards
    uninterrupted PE activity (pstate ramp).
"""

import contextlib
import math

import numpy as np

from concourse import bacc
import concourse.mybir as mybir
import concourse.tile as tile
from concourse.bass_utils import run_bass_kernel_spmd

F32 = mybir.dt.float32
F16 = mybir.dt.float16
BF16 = mybir.dt.bfloat16
I16 = mybir.dt.int16
EXP = mybir.ActivationFunctionType.Exp
AX = mybir.AxisListType.X
OP = mybir.AluOpType

S = 2048
E = 256
D = 64
NH = 4            # heads per core
NT = S // 128     # 16 r-tiles
SPAN = 1024       # scores psum span width

LN2 = math.log(2.0)
A_SCH = 128.0 / LN2          # Schraudolph scale
EXPBIAS = 127.0 * LN2        # 88.0297 = B/A; folded into the shift row
MARGIN = 40.0
MNEG = -60000.0

EXT = [S - 128 * t for t in range(NT)]
BASE = [0] * NT
for _t in range(1, NT):
    BASE[_t] = BASE[_t - 1] + EXT[_t - 1]
TOT = BASE[-1] + EXT[-1]


# ctx psum chunking: tiles per chunk (must keep L*65 <= 512). Heads 0-2 use
# big chunks (fewer normalize instructions); head 3 uses 2-tile chunks so the
# transpose/output tail pipelines into the span phase.
CHUNKS_STD = [(0, 7), (7, 7), (14, 2)]
CHUNKS_TAIL = [(0, 4), (4, 4), (8, 4), (12, 2), (14, 2)]

# scores span schedule per head: supercycles of 1024-wide ACT spans and
# 512-wide DVE (Schraudolph) spans. The DVE spans borrow a shared-pool bank,
# so PE effectively rotates through 3 score buffers: its runway per buffer
# reuse exceeds the exp chain latency and the PE never stalls on exp.
# Head 0 leans ACT (its phase is DVE-copy-heavy); heads 1-3 alternate to
# balance ACT and DVE against the available PE filler work.
def _make_spans(cycle):
    spans = []
    g = 0
    while g < TOT:
        for (w, eng) in cycle:
            if g >= TOT:
                break
            ln = min(w, TOT - g)
            if ln <= 512:
                # short trailing span -> DVE so head boundaries never put
                # two ACT spans back-to-back
                spans.append((g, ln, 512, "dve"))
            else:
                spans.append((g, ln, w, eng))
            g += ln
    return spans

SPANS_HN = _make_spans(((1024, "act"), (512, "dve")))
HEAD_SPANS = [SPANS_HN, SPANS_HN, SPANS_HN, SPANS_HN]


def build_nc():
    nc = bacc.Bacc(target_bir_lowering=False)

    xth = nc.declare_dram_parameter("xth", [128, 2, S], F16, isOutput=False)
    wqk2 = nc.declare_dram_parameter("wqk2", [128, 2, 2, 2, 128], F16,
                                     isOutput=False)
    wvi2 = nc.declare_dram_parameter("wvi2", [128, 2, NH * D], BF16, isOutput=False)
    wvo2 = nc.declare_dram_parameter("wvo2", [128, 2, E], BF16, isOutput=False)
    out = nc.declare_dram_parameter("out", [S, E], BF16, isOutput=True)

    with tile.TileContext(nc) as tc:
        _build(nc, tc, xth, wqk2, wvi2, wvo2, out)
    nc.finalize()
    return nc


def _build(nc, tc, xth, wqk2, wvi2, wvo2, out):
    ctx = contextlib.ExitStack()
    with ctx:
        const = ctx.enter_context(tc.tile_pool(name="const", bufs=1))
        persist = ctx.enter_context(tc.tile_pool(name="persist", bufs=1))
        work = ctx.enter_context(tc.tile_pool(name="work", bufs=3))
        etp = ctx.enter_context(tc.tile_pool(name="etp", bufs=2))
        stgp = ctx.enter_context(tc.tile_pool(name="stgp", bufs=2))
        ps_s = ctx.enter_context(tc.tile_pool(name="ps_s", bufs=2, space="PSUM"))
        ps_sh = ctx.enter_context(tc.tile_pool(name="ps_sh", bufs=3, space="PSUM"))
        ps_tx = ctx.enter_context(tc.tile_pool(name="ps_tx", bufs=1, space="PSUM"))

        # ---- inputs -> SBUF (weights first; SP queue head kept free) -------
        wqk_sb = persist.tile([128, 2, 2, 2, 128], F16, tag="wqk",
                              name="wqk_sb")
        nc.sync.dma_start(out=wqk_sb, in_=wqk2[:, :, :, :, :])
        xth_sb = persist.tile([128, 2, S], F16, tag="xth", name="xth_sb")
        for si in range(2):
            sp = slice(si * 1024, si * 1024 + 1024)
            nc.sync.dma_start(out=xth_sb[:, :, sp], in_=xth[:, :, sp])
        # xtb (bf16 copy of x^T for the V projection) is derived on-chip by
        # the mostly-idle Pool engine instead of a second HBM load
        xtb_sb = persist.tile([128, 2, S], BF16, tag="xtb", name="xtb_sb")
        wvi_sb = persist.tile([128, 2, NH * D], BF16, tag="wvi", name="wvi_sb")
        wvo_sb = persist.tile([128, 2, E], BF16, tag="wvo", name="wvo_sb")

        # ---- PE warm-up on zeros (overlaps input DMA; opens the pstate ramp)
        wz = const.tile([128, 128], BF16, tag="wz", name="wz")
        nc.gpsimd.memset(wz, 0.0)
        warm_sink = nc.dram_tensor("warm_sink", [1, 1], F32)
        pw = ps_s.tile([128, 1024], F32, tag="s", name="pw")
        for i in range(2):
            nc.tensor.matmul(pw[:, 0:128], wz, wz, start=(i == 0), stop=(i == 1),
                             skip_group_check=True)
        wsb = work.tile([1, 1], F32, tag="wsb", name="wsb")
        nc.vector.tensor_copy(wsb, pw[0:1, 0:1])
        nc.gpsimd.dma_start(out=warm_sink[:, :], in_=wsb)

        # small weights loaded after x so their transfers don't delay xth
        nc.scalar.dma_start(out=wvi_sb, in_=wvi2[:, :, :])
        nc.scalar.dma_start(out=wvo_sb, in_=wvo2[:, :, :])

        # ---- constants (Pool; ordered by first use) ------------------------
        # masktri64[r, c] = MNEG where c > r else 0 (keep c <= r)
        masktri = const.tile([128, 64], F16, tag="masktri", name="masktri")
        nc.gpsimd.memset(masktri, 0.0)
        nc.gpsimd.affine_select(
            out=masktri, in_=masktri, compare_op=OP.is_ge,
            fill=MNEG, base=0, pattern=[[-1, 64]], channel_multiplier=1)
        ident16 = const.tile([128, 128], F16, tag="ident16", name="ident16")
        nc.gpsimd.memset(ident16, 0.0)
        nc.gpsimd.affine_select(
            out=ident16, in_=ident16, compare_op=OP.not_equal,
            fill=1.0, base=0, pattern=[[-1, 128]], channel_multiplier=1)
        biasA = const.tile([128, 1], F32, tag="biasA", name="biasA")
        nc.gpsimd.memset(biasA, -EXPBIAS)
        for si in range(2):
            sp = slice(si * 1024, si * 1024 + 1024)
            nc.gpsimd.tensor_copy(xtb_sb[:, :, sp], xth_sb[:, :, sp])

        # ---- persistent per-head tensors -----------------------------------
        qp = [persist.tile([65, S], F16, tag=f"qp{n}", name=f"qp{n}")
              for n in range(NH)]
        kp = [persist.tile([65, S], F16, tag=f"kp{n}", name=f"kp{n}")
              for n in range(NH)]
        for n in range(2):
            nc.gpsimd.memset(qp[n][64:65, :], 1.0)
        # V' [128, NT, NH, 65]; col 64 = ones (denominator)
        vp = persist.tile([128, NT, NH, 65], BF16, tag="vp", name="vp")
        nc.gpsimd.memset(vp[:, :, :, 64:65], 1.0)
        identb = const.tile([128, 128], BF16, tag="identb", name="identb")
        nc.gpsimd.memset(identb, 0.0)
        nc.gpsimd.affine_select(
            out=identb, in_=identb, compare_op=OP.not_equal,
            fill=1.0, base=0, pattern=[[-1, 128]], channel_multiplier=1)
        for n in range(2, NH):
            nc.gpsimd.memset(qp[n][64:65, :], 1.0)
        # normalized-context transposed; plane p = head pair (2p, 2p+1)
        ctxT2 = persist.tile([128, 2, S], BF16, tag="ctxT2", name="ctxT2")
        # normalized context per pair: [128, NT, half, 64]
        cx2 = [persist.tile([128, NT, 2, D], BF16, tag=f"cx{p}", name=f"cx{p}")
               for p in range(2)]
        m_all = [work.tile([128, NT], F32, tag=f"m_all{n}", bufs=2,
                           name=f"m_all{n}") for n in range(NH)]

        ets = {}
        stages = {}

        # ---- QKV projections ----------------------------------------------
        def emit_qk_span(p, si):
            sp = slice(si * 512, si * 512 + 512)
            if si == 0:
                stages[p] = (
                    stgp.tile([128, S], F16, tag=f"stq{p}", name=f"stq{p}"),
                    stgp.tile([128, S], F16, tag=f"stk{p}", name=f"stk{p}"))
            stq, stk = stages[p]
            pq = ps_sh.tile([128, 512], F32, tag="sh", name="pq")
            for ec in range(2):
                nc.tensor.matmul(pq, wqk_sb[:, ec, 0, p, :],
                                 xth_sb[:, ec, sp],
                                 start=(ec == 0), stop=(ec == 1))
            nc.scalar.copy(stq[:, sp], pq)
            pk = ps_sh.tile([128, 512], F32, tag="sh", name="pk")
            for ec in range(2):
                nc.tensor.matmul(pk, wqk_sb[:, ec, 1, p, :],
                                 xth_sb[:, ec, sp],
                                 start=(ec == 0), stop=(ec == 1))
            nc.vector.tensor_copy(stk[:, sp], pk)
            if si == 3:
                # re-base the pair halves to per-head [65, S] tiles
                nc.sync.dma_start(out=qp[2 * p][0:64, :], in_=stq[0:64, :])
                nc.sync.dma_start(out=qp[2 * p + 1][0:64, :], in_=stq[64:128, :])
                nc.sync.dma_start(out=kp[2 * p][0:64, :], in_=stk[0:64, :])
                nc.sync.dma_start(out=kp[2 * p + 1][0:64, :], in_=stk[64:128, :])

        def emit_v_tile(t):
            cs = slice(t * 128, t * 128 + 128)
            pv = ps_sh.tile([128, 512], F32, tag="sh", name="pv")
            for ec in range(2):
                nc.tensor.matmul(pv[:, 0:256], xtb_sb[:, ec, cs],
                                 wvi_sb[:, ec, :],
                                 start=(ec == 0), stop=(ec == 1))
            cp = nc.scalar.copy if t % 2 == 0 else nc.vector.tensor_copy
            cp(vp[:, t, :, 0:64],
               pv[:, 0:256].rearrange("p (n d) -> p n d", d=64))

        # ---- prepass: sampled row maxes -> shift row of K' -----------------
        # reads the pair stage tiles directly (head n at partition 64*(n&1)),
        # so it does not wait for the per-head re-base DMAs.
        def emit_prepass_half(n, h):
            stq, stk = stages[n // 2]
            o = 64 * (n & 1)
            if n < 2:
                # pre-span phase: the scores pool is still idle
                pp = ps_s.tile([128, 1024], F32, tag="s", name="pp")[:, 0:512]
            else:
                pp = ps_sh.tile([128, 512], F32, tag="sh", name="pp")
            ppv = pp.rearrange("p (a b) -> p a b", b=64)
            for j in range(8):
                t = 8 * h + j
                rs = slice(t * 128, t * 128 + 128)
                if t == 0:
                    nc.tensor.matmul(ppv[:, j, :], ident16, masktri,
                                     start=True, stop=False,
                                     skip_group_check=True)
                    nc.tensor.matmul(ppv[:, j, :], stk[o:o + 64, rs],
                                     stq[o:o + 64, 0:64], start=False,
                                     stop=True, skip_group_check=True)
                else:
                    nc.tensor.matmul(ppv[:, j, :], stk[o:o + 64, rs],
                                     stq[o:o + 64, 0:64], start=True,
                                     stop=True)
            nc.vector.reduce_max(out=m_all[n][:, 8 * h:8 * h + 8], in_=ppv,
                                 axis=AX)

        def emit_prepass_tail(n):
            nst = work.tile([128, NT], F16, tag="nst", bufs=2, name="nst")
            nc.vector.tensor_scalar(
                out=nst, in0=m_all[n], scalar1=(MARGIN - EXPBIAS),
                scalar2=-1.0, op0=OP.add, op1=OP.mult)
            ptr = ps_tx.tile([16, 128], F16, tag="tx", name="ptr")
            nc.tensor.matmul(ptr, nst, ident16, is_transpose=True)
            stg = work.tile([16, 128], F16, tag="stg", bufs=2, name="stg")
            nc.vector.tensor_copy(stg, ptr)
            nc.sync.dma_start(
                out=kp[n][64:65, :].rearrange("p (t c) -> p t c", c=128),
                in_=stg)

        # ---- scores + exp --------------------------------------------------
        def g2piece(g):
            for t in range(NT):
                if g < BASE[t] + EXT[t]:
                    return t, g - BASE[t]
            raise AssertionError

        def emit_scores_span(n, si):
            et = ets[n]
            g0, ln, w, eng = HEAD_SPANS[n][si]
            if w == 1024:
                ps = ps_s.tile([128, 1024], F32, tag="s", name="ps")
            else:
                ps = ps_sh.tile([128, 512], F32, tag="sh", name="psc")
            g = g0
            while g < g0 + ln:
                t, off = g2piece(g)
                lc = min(512 - ((g - g0) % 512), BASE[t] + EXT[t] - g,
                         g0 + ln - g)
                cs = slice(t * 128, t * 128 + 128)
                nc.tensor.matmul(
                    ps[:, g - g0:g - g0 + lc], qp[n][:, cs],
                    kp[n][:, 128 * t + off:128 * t + off + lc],
                    start=True, stop=True)
                g += lc
            if eng == "dve":
                nc.vector.tensor_scalar(
                    out=et[:, g0:g0 + ln].bitcast(I16), in0=ps[:, 0:ln],
                    scalar1=0.0, scalar2=A_SCH, op0=OP.max, op1=OP.mult)
            else:
                nc.scalar.activation(
                    out=et[:, g0:g0 + ln], in_=ps[:, 0:ln], func=EXP,
                    bias=biasA)
            # zero the invalid (c > r) halves of diag blocks now complete
            for t in range(NT):
                if g0 < BASE[t] + 128 <= g0 + ln:
                    nc.gpsimd.affine_select(
                        out=et[:, BASE[t]:BASE[t] + 128],
                        in_=et[:, BASE[t]:BASE[t] + 128],
                        compare_op=OP.is_ge,
                        fill=0.0, base=0, pattern=[[1, 128]],
                        channel_multiplier=-1)

        # ---- context + normalize -------------------------------------------
        def make_ctx_state(n, chunks):
            return {"chunk": None, "pc": None, "chunks": chunks}

        def emit_ctx_tile(n, t, st):
            et = ets[n]
            chunks = st["chunks"]
            ci = next(i for i, (t0, L) in enumerate(chunks)
                      if t0 <= t < t0 + L)
            t0, L = chunks[ci]
            if st["chunk"] != ci:
                st["chunk"] = ci
                pc = ps_sh.tile([128, 512], F32, tag="sh", name="pc")
                st["pcv"] = pc[:, 0:65 * L].rearrange("p (a b) -> p a b", b=65)
            pcv = st["pcv"]
            for u in range(t + 1):
                g = BASE[u] + 128 * (t - u)
                nc.tensor.matmul(pcv[:, t - t0, :], et[:, g:g + 128],
                                 vp[:, u, n, :],
                                 start=(u == 0), stop=(u == t))
            if t == t0 + L - 1:
                pc3 = pcv[:, 0:L, :]
                rcp = work.tile([128, 7, 1], F32, tag="rcp", bufs=3,
                                name="rcp")
                nc.vector.reciprocal(rcp[:, 0:L, :], pc3[:, :, 64:65])
                nc.vector.tensor_tensor(
                    out=cx2[n // 2][:, t0:t0 + L, n % 2, :],
                    in0=pc3[:, :, 0:64],
                    in1=rcp[:, 0:L, :].broadcast_to([128, L, 64]),
                    op=OP.mult)

        # ---- pair transpose + fused output projection ----------------------
        def emit_tx_group(p, t0, L, fuse_out=False, late=False):
            # transpose tiles [t0, t0+L) of pair p into ctxT2. After the last
            # scores span the ps_s banks are idle: late groups borrow them.
            if late:
                ptx = ps_s.tile([128, 1024], F32, tag="s",
                                name="ptxl").bitcast(BF16)[:, 0:512]
                ptx = ptx.rearrange("p (a b) -> p a b", b=128)[:, 0:L, :]
            else:
                ptx = ps_tx.tile([128, 4, 128], BF16, tag="tx",
                                 name="ptx")[:, 0:L, :]
            for j in range(L):
                t = t0 + j
                nc.tensor.matmul(
                    ptx[:, j, :],
                    cx2[p][:, t, :, :].rearrange("p a b -> p (a b)"),
                    identb, is_transpose=True, skip_group_check=True)
            sp = slice(t0 * 128, (t0 + L) * 128)
            nc.vector.tensor_copy(ctxT2[:, p, sp],
                                  ptx.rearrange("p a b -> p (a b)"))
            if fuse_out and late:
                po4 = ps_s.tile([128, 1024], F32, tag="s", name="po4")
                po4v = po4.rearrange("p (a b) -> p a b", b=256)
                for j in range(L):
                    ts = slice((t0 + j) * 128, (t0 + j) * 128 + 128)
                    for g in range(2):
                        nc.tensor.matmul(po4v[:, j, :], ctxT2[:, g, ts],
                                         wvo_sb[:, g, :],
                                         start=(g == 0), stop=(g == 1),
                                         skip_group_check=True)
                osb = work.tile([128, 4, E], BF16, tag="osb", bufs=3,
                                name="osb")
                nc.scalar.copy(osb[:, 0:L, :], po4v[:, 0:L, :])
                nc.sync.dma_start(
                    out=out[t0 * 128:(t0 + L) * 128, :].rearrange(
                        "(t p) e -> p t e", p=128),
                    in_=osb[:, 0:L, :])
            elif fuse_out:
                osb = work.tile([128, 4, E], BF16, tag="osb", bufs=3,
                                name="osb")
                for j in range(L):
                    t = t0 + j
                    ts = slice(t * 128, t * 128 + 128)
                    po = ps_sh.tile([128, 512], F32, tag="sh", name="po")
                    for g in range(2):
                        nc.tensor.matmul(po[:, 0:256], ctxT2[:, g, ts],
                                         wvo_sb[:, g, :],
                                         start=(g == 0), stop=(g == 1))
                    ocp = nc.scalar.copy if j % 2 == 0 else \
                        nc.vector.tensor_copy
                    ocp(osb[:, j, :], po[:, 0:256])
                nc.sync.dma_start(
                    out=out[t0 * 128:(t0 + L) * 128, :].rearrange(
                        "(t p) e -> p t e", p=128),
                    in_=osb[:, 0:L, :])

        # ================= emission schedule ================================
        # upfront: pair0 projections + prepass(0..1); prepass reads stages so
        # only the scatter DMA (shift row) gates scores(0). Prepass half h
        # needs only the first/second half of the stages, so it interleaves
        # with the qk spans as their xth chunks arrive.
        for si in range(4):
            emit_qk_span(0, si)
        emit_prepass_half(0, 0)
        emit_prepass_half(0, 1)
        emit_prepass_tail(0)
        emit_prepass_half(1, 0)
        emit_prepass_half(1, 1)
        emit_prepass_tail(1)

        # filler queues for each scores head: (pre_count, closures)
        def head_fillers(n):
            f = []
            pre = 0
            if n == 0:
                # pair1 projections + first v tiles keep the PE fed while
                # scores(0) buffers rotate
                for si in range(4):
                    f.append(lambda si=si: emit_qk_span(1, si))
                for t in range(8):
                    f.append(lambda t=t: emit_v_tile(t))
                pre = 4
            elif n == 1:
                st = make_ctx_state(0, CHUNKS_STD)
                f.append(lambda: emit_prepass_half(2, 0))
                f.append(lambda: emit_prepass_half(2, 1))
                f.append(lambda: emit_prepass_tail(2))
                for t in range(NT):
                    if t + 8 < NT:
                        f.append(lambda t=t: emit_v_tile(t + 8))
                    f.append(lambda t=t, st=st: emit_ctx_tile(0, t, st))
                    if t == 3:
                        f.append(lambda: emit_prepass_half(3, 0))
                        f.append(lambda: emit_prepass_half(3, 1))
                        f.append(lambda: emit_prepass_tail(3))
            else:
                st = make_ctx_state(n - 1, CHUNKS_STD)
                for t in range(NT):
                    f.append(lambda t=t, st=st: emit_ctx_tile(n - 1, t, st))
                if n == 2:
                    # pair0 transposes (ctx(1) normalize done mid-head-2)
                    for t4 in range(4):
                        f.append(lambda t4=t4: emit_tx_group(0, 4 * t4, 4))
            return pre, f

        # min head-3 span index after which ctx(3) tile t's et columns exist
        # (+lag so the PE does not catch the exp wavefront)
        def ctx3_ready_span(t):
            need = BASE[t] + 128 if t < NT - 1 else TOT
            for si, (g0, ln, w, eng) in enumerate(HEAD_SPANS[3]):
                if g0 + ln >= need:
                    return min(si + 3, len(HEAD_SPANS[3]) - 1)
            return len(HEAD_SPANS[3]) - 1

        st3 = make_ctx_state(3, CHUNKS_TAIL)
        tail_state = {"t": 0}

        def pump_tail(max_span_done):
            # emit ready ctx(3) tiles + finished chunks' tx/outproj groups
            while tail_state["t"] < NT and \
                    ctx3_ready_span(tail_state["t"]) <= max_span_done:
                t = tail_state["t"]
                emit_ctx_tile(3, t, st3)
                tail_state["t"] += 1
                for ci, (t0, L) in enumerate(CHUNKS_TAIL):
                    if t == t0 + L - 1:
                        emit_tx_group(1, t0, L, fuse_out=True,
                                      late=(t0 >= 8))

        # heads 1-3 have their first CROSS spans emitted as late fillers of
        # the previous head, smoothing the boundary (the exp may briefly wait
        # for the et buffer, but the PE keeps streaming)
        CROSS = 3
        for n in range(NH):
            if n == 0:
                ets[n] = etp.tile([128, TOT], BF16, tag="et", name=f"et{n}")
            pre, fillers = head_fillers(n)
            if n < NH - 1:
                def open_next(nn=n + 1):
                    ets[nn] = etp.tile([128, TOT], BF16, tag="et",
                                       name=f"et{nn}")
                fillers.append(open_next)
                for sj in range(CROSS):
                    fillers.append(
                        lambda nn=n + 1, sj=sj: emit_scores_span(nn, sj))
            fi = 0
            while fi < pre:
                fillers[fi]()
                fi += 1
            # distribute fillers across spans, weighted to the span count
            nspan = len(HEAD_SPANS[n]) - (0 if n == 0 else CROSS)
            per = max(1, (len(fillers) - fi + nspan - 1) // nspan)
            for si in range(0 if n == 0 else CROSS, len(HEAD_SPANS[n])):
                emit_scores_span(n, si)
                stop_at = min(len(fillers), fi + per)
                while fi < stop_at:
                    fillers[fi]()
                    fi += 1
                if n == 3:
                    pump_tail(si)
            while fi < len(fillers):
                fillers[fi]()
                fi += 1
            if n >= 2:
                ets.pop(n - 2)

        # whatever the pump did not drain
        pump_tail(len(HEAD_SPANS[3]) - 1)
        ets.pop(2)
        ets.pop(3)


_NC_CACHE = None


def kernel(x, key_matrices, query_matrices, value_in_matrices, value_out_matrices):
    global _NC_CACHE
    import ml_dtypes

    x = np.asarray(x, dtype=np.float32)
    wk_full = np.asarray(key_matrices, dtype=np.float32)
    wq_full = np.asarray(query_matrices, dtype=np.float32)
    wvi_full = np.asarray(value_in_matrices, dtype=np.float32)
    wvo_full = np.asarray(value_out_matrices, dtype=np.float32)
    B = x.shape[0]

    in_maps = []
    for core in range(8):
        b, g = core % 4, core // 4
        hs = slice(g * NH, g * NH + NH)
        xt = np.ascontiguousarray(x[b].T)                       # [E, S]
        xt3 = xt.reshape(2, 128, S).transpose(1, 0, 2)          # [128, 2, S]
        wq = wq_full[hs]                                        # [4, E, D]
        wk = wk_full[hs]
        wvi = wvi_full[hs]
        wvo = wvo_full[hs]                                      # [4, D, E]
        # [128, 2ec, 2pair, 128(A|B)]
        wq2 = np.stack([
            np.stack([np.concatenate([wq[2 * p][ec * 128:(ec + 1) * 128],
                                      wq[2 * p + 1][ec * 128:(ec + 1) * 128]],
                                     axis=1) for p in range(2)], axis=0)
            for ec in range(2)], axis=0).transpose(2, 0, 1, 3)
        wk2 = np.stack([
            np.stack([np.concatenate([wk[2 * p][ec * 128:(ec + 1) * 128],
                                      wk[2 * p + 1][ec * 128:(ec + 1) * 128]],
                                     axis=1) for p in range(2)], axis=0)
            for ec in range(2)], axis=0).transpose(2, 0, 1, 3)
        # [128, 2ec, 4n*64d]
        wvi2 = np.stack([wvi[:, ec * 128:(ec + 1) * 128, :]
                         .transpose(1, 0, 2).reshape(128, NH * D)
                         for ec in range(2)], axis=1)
        # [128(=64*(n&1)+d), 2plane, 256]
        wvo2 = np.stack([np.concatenate([wvo[2 * p], wvo[2 * p + 1]], axis=0)
                         for p in range(2)], axis=1)
        wqk2 = np.stack([wq2, wk2], axis=2)   # [128, 2ec, 2qk, 2pair, 128]
        in_maps.append({
            "xth": np.ascontiguousarray(xt3).astype(np.float16),
            "wqk2": np.ascontiguousarray(wqk2).astype(np.float16),
            "wvi2": np.ascontiguousarray(wvi2).astype(ml_dtypes.bfloat16),
            "wvo2": np.ascontiguousarray(wvo2).astype(ml_dtypes.bfloat16),
        })

    if _NC_CACHE is None:
        _NC_CACHE = build_nc()
    res = run_bass_kernel_spmd(_NC_CACHE, in_maps, core_ids=list(range(8)))
    outs = res.results if hasattr(res, "results") else res

    full = np.zeros((B, S, E), dtype=np.float32)
    for core in range(8):
        full[core % 4] += outs[core]["out"].astype(np.float32)
    return full
